# revision 14
# baseline (speedup 1.0000x reference)
"""LipschitzRNN Trainium2 kernel — correction-chain design.

Math (per reference):
    A = (1-bA)(MA+MA.T) + bA(MA-MA.T) - YA*I ; C likewise with bW mix
    X_{t+1} = X_t + STEP*(A@X_t + tanh(C@X_t + by))
    out[b, t, :] = X_t[:, b]

Device strategy (8-way batch data-parallel, 32 cols/core, no collectives):
  State z = X/STEP in fp16 (STEP-scaled weights keep fp16 relative error;
  the tanh term enters unscaled; fp16 carry random-walks to ~9.5e-3 rel,
  budget 2e-2).  The serial per-step chain is the wall-clock limit
  (engines are mostly idle), so the tanh is taken OFF the chain by a
  first-order Taylor correction:

      Yhat_j = by + (Cs D)@z_{j-1}          (one step of slack; D = I+Ah)
      That_j = tanh(Yhat_j),  S_j = 1 - That_j^2
      c_j    = (Cs@That_{j-1}) * S_j        (first-order delta correction)
      z_j    = z_{j-1} + Ah@z_{j-1} + That_{j-1} [+ c_{j-4}, injected late]

  with Ah = STEP*A, Cs = STEP*C (fp16), CsD = f16(STEP*C@(I+STEP*A)).
  The only true serial chain is  z -> {I,Ah}-matmuls -> CAST -> z
  (~700ns); tanh/Square/S/c ride parallel tracks with >=1 step slack, and
  the tiny correction (|c|~0.05 in z units ~ 1e-4 of X) enters 3 steps
  late, which the numpy model shows costs nothing (rel err 9.53e-3).
  Bias is folded into the Yhat bank with padded-128 "bias weight"
  matmuls (row 0 = by chunk) against a constant e0 tile, so a single
  [128,64] tanh instruction needs no per-partition bias and the PE never
  switches tile configs (k=1 matmuls cost ~100ns reconfig stalls).
  Output: z blocks [128, 16 steps, 64] DMA'd raw every 16 steps
  (2KB/descriptor); host transposes [p,t,c*32+b] -> [b,t,n] and scales
  by STEP.  No PE transposes, no staging copies.
"""

import numpy as np

N = 256
BS = 256
TMAX = 512
STEP = 0.01
YA = 0.001
YW = 0.001
NCORES = 8
BLOC = BS // NCORES      # 32 batch cols per core
GRP = 16                 # output steps per DMA block
NSTEPS = TMAX - 1        # 511
CLAG = 4                 # c_{j-CLAG} injected into zb_j

LAST_RESULT = None  # BassKernelResults of the most recent run (for test harness)


def _build(n_steps):
    from concourse import bacc, tile
    import concourse.mybir as mybir
    from concourse.masks import make_identity

    F32 = mybir.dt.float32
    F16 = mybir.dt.float16
    AF = mybir.ActivationFunctionType
    ALU = mybir.AluOpType

    nc = bacc.Bacc("TRN2", target_bir_lowering=False, debug=False,
                   num_devices=NCORES)

    # all weights/constants/init state packed into one tensor: a single
    # input DMA instead of 21 serialized ~650ns ones (saves ~12us startup)
    NPACK = 19 * 128 + BLOC + 4 * BLOC
    WPACK = nc.dram_tensor("WPACK", [128, NPACK], F16, kind="ExternalInput")
    OUT = nc.dram_tensor("OUT", [128, n_steps, 2 * BLOC], F16,
                         kind="ExternalOutput")

    W = 2 * BLOC  # 64: working tile width (2 n-chunks x 32 batch)

    with tile.TileContext(nc) as tc:
        with (
            tc.tile_pool(name="consts", bufs=1) as consts,
            tc.tile_pool(name="tpool", bufs=4) as tpool,
            tc.tile_pool(name="qpool", bufs=3) as qpool,
            tc.tile_pool(name="spool", bufs=2) as spool,
            tc.tile_pool(name="cpool", bufs=CLAG + 3) as cpool,
            tc.tile_pool(name="stg", bufs=2) as stgp,
            tc.tile_pool(name="zb", bufs=1, space="PSUM") as zbp,
            tc.tile_pool(name="yb", bufs=3, space="PSUM") as ybp,
            tc.tile_pool(name="db", bufs=3, space="PSUM") as dbp,
        ):
            # ---- constants / initial state (one packed DMA) ----
            wpk = consts.tile([128, NPACK], F16, tag="wpk", name="wpk")
            nc.sync.dma_start(wpk[:], WPACK[:, :])

            def wsl(i):
                return wpk[:, i * 128:(i + 1) * 128]
            wa = [[wsl(0 + 2 * k + m) for m in range(2)] for k in range(2)]
            wcs = [[wsl(4 + 2 * k + m) for m in range(2)] for k in range(2)]
            wcd = [[wsl(8 + 2 * k + m) for m in range(2)] for k in range(2)]
            wcd2 = [[wsl(12 + 2 * k + m) for m in range(2)] for k in range(2)]
            bp = [wsl(16), wsl(17)]
            ident = wsl(18)
            e0 = wpk[:, 2432:2432 + BLOC]
            z0t = wpk[:, 2464:2464 + 2 * BLOC]
            t0t = wpk[:, 2528:2528 + 2 * BLOC]

            def apply_mat(bank, wt, src_, start, stop):
                for m in range(2):
                    o = bank[:, m * BLOC:(m + 1) * BLOC]
                    nc.tensor.matmul(o, wt[0][m], src_[:, 0:BLOC],
                                     start=start, stop=False)
                    nc.tensor.matmul(o, wt[1][m], src_[:, BLOC:W],
                                     start=False, stop=stop)

            def bias_mms(yb):
                nc.tensor.matmul(yb[:, 0:BLOC], bp[0], e0,
                                 start=True, stop=False)
                nc.tensor.matmul(yb[:, BLOC:W], bp[1], e0,
                                 start=True, stop=False)

            # ---- persistent f32 z-bank: init M = I@z0, then accumulate
            # Ah@z + That + c onto it forever (exact f32 carry) ----
            zb = zbp.tile([128, W], F32, tag="zbk", name="zbk")
            nc.tensor.matmul(zb[:, 0:W], ident, z0t[:, 0:W],
                             start=True, stop=False, skip_group_check=True)

            t_jm1 = t0t
            t_m2 = None             # That_{j-2}
            q_jm1 = None            # Q_{j-1}
            db_jm1 = None           # delta-hat bank from prev iter
            cql = []                # pending c tiles (FIFO)
            z = z0t
            z_m2 = None             # z_{j-2}
            stg = None

            for j in range(1, n_steps + 1):
                s = (j - 1) % GRP
                t0i = j - 1 - s
                blk = min(GRP, n_steps - t0i)

                # ---- z bank accumulate: I@That_{j-2} [+ I@c] + Ah@z ----
                # (That joins z via the STT below and enters the bank one
                # step later, so every bank matmul input is ready early and
                # the block is gated only by the bank WAR on the STT)
                if j >= 2:
                    nc.tensor.matmul(zb[:, 0:W], ident, t_m2[:, 0:W],
                                     start=False, stop=False,
                                     skip_group_check=True)
                if j > CLAG:
                    cinj = cql.pop(0)
                    nc.tensor.matmul(zb[:, 0:W], ident, cinj[:, 0:W],
                                     start=False, stop=False,
                                     skip_group_check=True)
                for m in range(2):
                    o = zb[:, m * BLOC:(m + 1) * BLOC]
                    nc.tensor.matmul(o, wa[0][m], z[:, 0:BLOC],
                                     start=False, stop=False,
                                     skip_group_check=True)
                    nc.tensor.matmul(o, wa[1][m], z[:, BLOC:W],
                                     start=False, stop=(m == 1),
                                     skip_group_check=True)

                # ---- Yhat_j bank + That_j + Q_j ----
                # j>=2: by + CsD2@z_{j-2} + CsD@That_{j-2}   (2-step slack)
                t_j = q_j = None
                if j < n_steps:
                    yb = ybp.tile([128, W], F32, tag="yb", name="yb")
                    bias_mms(yb)
                    if j == 1:
                        apply_mat(yb, wcd, z, start=False, stop=True)
                    else:
                        apply_mat(yb, wcd2, z_m2, start=False, stop=False)
                        apply_mat(yb, wcd, t_m2, start=False, stop=True)
                    t_j = tpool.tile([128, W], F16, tag="tt", name="tt")
                    nc.scalar.activation(t_j[:], yb[:], AF.Tanh)
                    q_j = qpool.tile([128, W], F16, tag="qq", name="qq")
                    nc.scalar.activation(q_j[:], t_j[:], AF.Square)

                # ---- delta-hat bank: Cs @ That_{j-1} ----
                db = dbp.tile([128, W], F32, tag="dbk", name="dbk")
                for m in range(2):
                    o = db[:, m * BLOC:(m + 1) * BLOC]
                    nc.tensor.matmul(o, wcs[0][m], t_jm1[:, 0:BLOC],
                                     start=True, stop=False)
                    nc.tensor.matmul(o, wcs[1][m], t_jm1[:, BLOC:W],
                                     start=False, stop=True)

                # ---- CAST: z_j = zb -> fp16, straight into DMA staging ----
                if s == 0:
                    stg = stgp.tile([128, GRP, W], F16, tag="stg", name="stg")
                zn = stg[:, s, :]
                nc.vector.scalar_tensor_tensor(
                    zn, zb[:], 1.0, t_jm1[:, 0:W], op0=ALU.mult, op1=ALU.add)

                # ---- S_{j-1} and c_{j-1} (one iteration late, off-chain) ----
                if 2 <= j <= n_steps - CLAG + 1:
                    ss = spool.tile([128, W], F16, tag="ss", name="ss")
                    nc.vector.tensor_scalar(ss[:], q_jm1[:], -1.0, 1.0,
                                            op0=ALU.mult, op1=ALU.add)
                    cc = cpool.tile([128, W], F16, tag="cc", name="cc")
                    nc.vector.scalar_tensor_tensor(
                        cc[:], db_jm1[:], 1.0, ss[:],
                        op0=ALU.mult, op1=ALU.mult)
                    cql.append(cc)

                if s == blk - 1:
                    nc.sync.dma_start(OUT[:, t0i:t0i + blk, :],
                                      stg[:, 0:blk, :])

                z_m2 = z
                z = zn
                t_m2 = t_jm1
                t_jm1 = t_j
                q_jm1 = q_j
                db_jm1 = db
    nc.compile()
    return nc


def kernel(X0, MA, MW, bA_z, bW_z, by_w):
    global LAST_RESULT
    from concourse.bass_utils import run_bass_kernel_spmd

    X0 = np.asarray(X0, dtype=np.float32)
    MA = np.asarray(MA, dtype=np.float32)
    MW = np.asarray(MW, dtype=np.float32)
    bA_z = np.asarray(bA_z, dtype=np.float32)
    bW_z = np.asarray(bW_z, dtype=np.float32)
    by_w = np.asarray(by_w, dtype=np.float32)

    bA = np.float32(0.5) * np.exp(-bA_z[0, 0] * bA_z[0, 0]) + np.float32(0.5)
    bW = np.float32(0.5) * np.exp(-bW_z[0, 0] * bW_z[0, 0]) + np.float32(0.5)
    I = np.eye(N, dtype=np.float32)
    A = (1 - bA) * (MA + MA.T) + bA * (MA - MA.T) - np.float32(YA) * I
    C = (1 - bA) * (MW + MW.T) + bW * (MW - MW.T) - np.float32(YW) * I

    f16 = lambda x: x.astype(np.float16).astype(np.float32)
    Ah = f16(np.float32(STEP) * A)
    Cs = f16(np.float32(STEP) * C)
    Dm = I + np.float32(STEP) * A
    CsD = f16((np.float32(STEP) * C) @ Dm)
    CsD2 = f16((np.float32(STEP) * C) @ Dm @ Dm)
    byh = f16(by_w)

    NPACK = 19 * 128 + BLOC + 4 * BLOC
    base = np.zeros((128, NPACK), dtype=np.float16)
    for wi, M in enumerate([Ah, Cs, CsD, CsD2]):
        MT = M.T.astype(np.float16)
        for k in range(2):
            for m in range(2):
                idx = 4 * wi + 2 * k + m
                base[:, idx * 128:(idx + 1) * 128] = \
                    MT[128 * k:128 * (k + 1), 128 * m:128 * (m + 1)]
    base[0, 16 * 128:16 * 128 + 128] = byh[0:128, 0].astype(np.float16)
    base[0, 17 * 128:17 * 128 + 128] = byh[128:256, 0].astype(np.float16)
    base[:, 18 * 128:19 * 128] = np.eye(128, dtype=np.float16)
    base[0, 2432:2432 + BLOC] = 1.0

    in_maps = []
    for i in range(NCORES):
        Xc = X0[i * BLOC:(i + 1) * BLOC, :] / np.float32(STEP)   # [32, 256]
        z0 = Xc.T.astype(np.float16)                              # [256, 32]
        z0f = z0.astype(np.float32)
        T0f = np.tanh(Cs @ z0f + byh).astype(np.float16)          # [256, 32]
        WPACKh = base.copy()
        for c in range(2):
            WPACKh[:, 2464 + c * BLOC:2464 + (c + 1) * BLOC] = \
                z0[c * 128:(c + 1) * 128, :]
            WPACKh[:, 2528 + c * BLOC:2528 + (c + 1) * BLOC] = \
                T0f[c * 128:(c + 1) * 128, :]
        in_maps.append({"WPACK": WPACKh})

    nc = _build(NSTEPS)
    res = run_bass_kernel_spmd(nc, in_maps, core_ids=list(range(NCORES)))
    LAST_RESULT = res

    out = np.empty((BS, TMAX, N), dtype=np.float32)
    out[:, 0, :] = X0
    for i in range(NCORES):
        O = np.asarray(res.results[i]["OUT"]).reshape(128, NSTEPS, 2, BLOC)
        # [p, t, c, b] -> [b, t, c, p] -> [32, 511, 256]
        blockX = O.transpose(3, 1, 2, 0).reshape(BLOC, NSTEPS, N)
        out[i * BLOC:(i + 1) * BLOC, 1:, :] = (
            blockX.astype(np.float32) * np.float32(STEP))
    return out


if __name__ == "__main__":
    rng = np.random.default_rng(0)
    inputs = {
        "X0": rng.standard_normal((BS, N), dtype=np.float32),
        "MA": rng.standard_normal((N, N), dtype=np.float32) / 16,
        "MW": rng.standard_normal((N, N), dtype=np.float32) / 16,
        "bA_z": np.full((1, 1), 0.65, dtype=np.float32),
        "bW_z": np.full((1, 1), 0.65, dtype=np.float32),
        "by_w": rng.standard_normal((N, 1), dtype=np.float32) / 100,
    }
    out = kernel(**inputs)
    print("out", out.shape, out.dtype, np.abs(out).max())


# revision 15
# speedup vs baseline: 1.0381x; 1.0381x over previous
"""LipschitzRNN Trainium2 kernel — correction-chain design.

Math (per reference):
    A = (1-bA)(MA+MA.T) + bA(MA-MA.T) - YA*I ; C likewise with bW mix
    X_{t+1} = X_t + STEP*(A@X_t + tanh(C@X_t + by))
    out[b, t, :] = X_t[:, b]

Device strategy (8-way batch data-parallel, 32 cols/core, no collectives):
  State z = X/STEP in fp16 (STEP-scaled weights keep fp16 relative error;
  the tanh term enters unscaled; fp16 carry random-walks to ~9.5e-3 rel,
  budget 2e-2).  The serial per-step chain is the wall-clock limit
  (engines are mostly idle), so the tanh is taken OFF the chain by a
  first-order Taylor correction:

      Yhat_j = by + (Cs D)@z_{j-1}          (one step of slack; D = I+Ah)
      That_j = tanh(Yhat_j),  S_j = 1 - That_j^2
      c_j    = (Cs@That_{j-1}) * S_j        (first-order delta correction)
      z_j    = z_{j-1} + Ah@z_{j-1} + That_{j-1} [+ c_{j-4}, injected late]

  with Ah = STEP*A, Cs = STEP*C (fp16), CsD = f16(STEP*C@(I+STEP*A)).
  The only true serial chain is  z -> {I,Ah}-matmuls -> CAST -> z
  (~700ns); tanh/Square/S/c ride parallel tracks with >=1 step slack, and
  the tiny correction (|c|~0.05 in z units ~ 1e-4 of X) enters 3 steps
  late, which the numpy model shows costs nothing (rel err 9.53e-3).
  Bias is folded into the Yhat bank with padded-128 "bias weight"
  matmuls (row 0 = by chunk) against a constant e0 tile, so a single
  [128,64] tanh instruction needs no per-partition bias and the PE never
  switches tile configs (k=1 matmuls cost ~100ns reconfig stalls).
  Output: z blocks [128, 16 steps, 64] DMA'd raw every 16 steps
  (2KB/descriptor); host transposes [p,t,c*32+b] -> [b,t,n] and scales
  by STEP.  No PE transposes, no staging copies.
"""

import numpy as np

N = 256
BS = 256
TMAX = 512
STEP = 0.01
YA = 0.001
YW = 0.001
NCORES = 8
BLOC = BS // NCORES      # 32 batch cols per core
GRP = 16                 # output steps per DMA block
NSTEPS = TMAX - 1        # 511
CLAG = 4                 # c_{j-CLAG} injected into zb_j

LAST_RESULT = None  # BassKernelResults of the most recent run (for test harness)


def _build(n_steps):
    from concourse import bacc, tile
    import concourse.mybir as mybir
    from concourse.masks import make_identity

    F32 = mybir.dt.float32
    F16 = mybir.dt.float16
    AF = mybir.ActivationFunctionType
    ALU = mybir.AluOpType

    nc = bacc.Bacc("TRN2", target_bir_lowering=False, debug=False,
                   num_devices=NCORES)

    # all weights/constants/init state packed into one tensor: a single
    # input DMA instead of 21 serialized ~650ns ones (saves ~12us startup)
    NPACK = 19 * 128 + BLOC + 4 * BLOC
    WPACK = nc.dram_tensor("WPACK", [128, NPACK], F16, kind="ExternalInput")
    OUT = nc.dram_tensor("OUT", [128, n_steps, 2 * BLOC], F16,
                         kind="ExternalOutput")

    W = 2 * BLOC  # 64: working tile width (2 n-chunks x 32 batch)

    with tile.TileContext(nc) as tc:
        with (
            tc.tile_pool(name="consts", bufs=1) as consts,
            tc.tile_pool(name="tpool", bufs=4) as tpool,
            tc.tile_pool(name="qpool", bufs=3) as qpool,
            tc.tile_pool(name="spool", bufs=2) as spool,
            tc.tile_pool(name="cpool", bufs=CLAG + 3) as cpool,
            tc.tile_pool(name="stg", bufs=2) as stgp,
            tc.tile_pool(name="zb", bufs=1, space="PSUM") as zbp,
            tc.tile_pool(name="yb", bufs=3, space="PSUM") as ybp,
            tc.tile_pool(name="db", bufs=3, space="PSUM") as dbp,
        ):
            # ---- constants / initial state (one packed DMA) ----
            wpk = consts.tile([128, NPACK], F16, tag="wpk", name="wpk")
            nc.sync.dma_start(wpk[:], WPACK[:, :])

            def wsl(i):
                return wpk[:, i * 128:(i + 1) * 128]
            wa = [[wsl(0 + 2 * k + m) for m in range(2)] for k in range(2)]
            wcs = [[wsl(4 + 2 * k + m) for m in range(2)] for k in range(2)]
            wcd = [[wsl(8 + 2 * k + m) for m in range(2)] for k in range(2)]
            wcd2 = [[wsl(12 + 2 * k + m) for m in range(2)] for k in range(2)]
            bp = [wsl(16), wsl(17)]
            ident = wsl(18)
            e0 = wpk[:, 2432:2432 + BLOC]
            z0t = wpk[:, 2464:2464 + 2 * BLOC]
            t0t = wpk[:, 2528:2528 + 2 * BLOC]

            def apply_mat(bank, wt, src_, start, stop):
                for m in range(2):
                    o = bank[:, m * BLOC:(m + 1) * BLOC]
                    nc.tensor.matmul(o, wt[0][m], src_[:, 0:BLOC],
                                     start=start, stop=False)
                    nc.tensor.matmul(o, wt[1][m], src_[:, BLOC:W],
                                     start=False, stop=stop)

            def bias_mms(yb):
                nc.tensor.matmul(yb[:, 0:BLOC], bp[0], e0,
                                 start=True, stop=False)
                nc.tensor.matmul(yb[:, BLOC:W], bp[1], e0,
                                 start=True, stop=False)

            # ---- persistent f32 z-bank: init M = I@z0, then accumulate
            # Ah@z + That + c onto it forever (exact f32 carry) ----
            zb = zbp.tile([128, W], F32, tag="zbk", name="zbk")
            nc.tensor.matmul(zb[:, 0:W], ident, z0t[:, 0:W],
                             start=True, stop=False, skip_group_check=True)

            t_jm1 = t0t
            t_m2 = None             # That_{j-2}
            q_jm1 = None            # Q_{j-1}
            db_jm1 = None           # delta-hat bank from prev iter
            cql = []                # pending c tiles (FIFO)
            z = z0t
            z_m2 = None             # z_{j-2}
            stg = None

            for j in range(1, n_steps + 1):
                s = (j - 1) % GRP
                t0i = j - 1 - s
                blk = min(GRP, n_steps - t0i)

                # ---- z bank accumulate: + Ah@z [+ I@c] + I@That_{j-1} ----
                # (I@That last: That_{j-1} is the latest-arriving input)
                for m in range(2):
                    o = zb[:, m * BLOC:(m + 1) * BLOC]
                    nc.tensor.matmul(o, wa[0][m], z[:, 0:BLOC],
                                     start=False, stop=False,
                                     skip_group_check=True)
                    nc.tensor.matmul(o, wa[1][m], z[:, BLOC:W],
                                     start=False, stop=False,
                                     skip_group_check=True)
                if j > CLAG:
                    cinj = cql.pop(0)
                    nc.tensor.matmul(zb[:, 0:W], ident, cinj[:, 0:W],
                                     start=False, stop=False,
                                     skip_group_check=True)
                nc.tensor.matmul(zb[:, 0:W], ident, t_jm1[:, 0:W],
                                 start=False, stop=True, skip_group_check=True)

                # ---- Yhat_j bank + That_j + Q_j ----
                # j>=2: by + CsD2@z_{j-2} + CsD@That_{j-2}   (2-step slack)
                t_j = q_j = None
                if j < n_steps:
                    yb = ybp.tile([128, W], F32, tag="yb", name="yb")
                    bias_mms(yb)
                    if j == 1:
                        apply_mat(yb, wcd, z, start=False, stop=True)
                    else:
                        apply_mat(yb, wcd2, z_m2, start=False, stop=False)
                        apply_mat(yb, wcd, t_m2, start=False, stop=True)
                    t_j = tpool.tile([128, W], F16, tag="tt", name="tt")
                    nc.scalar.activation(t_j[:], yb[:], AF.Tanh)
                    q_j = qpool.tile([128, W], F16, tag="qq", name="qq")
                    nc.scalar.activation(q_j[:], t_j[:], AF.Square)

                # ---- delta-hat bank: Cs @ That_{j-1} ----
                db = dbp.tile([128, W], F32, tag="dbk", name="dbk")
                for m in range(2):
                    o = db[:, m * BLOC:(m + 1) * BLOC]
                    nc.tensor.matmul(o, wcs[0][m], t_jm1[:, 0:BLOC],
                                     start=True, stop=False)
                    nc.tensor.matmul(o, wcs[1][m], t_jm1[:, BLOC:W],
                                     start=False, stop=True)

                # ---- CAST: z_j = zb -> fp16, straight into DMA staging ----
                if s == 0:
                    stg = stgp.tile([128, GRP, W], F16, tag="stg", name="stg")
                zn = stg[:, s, :]
                nc.vector.tensor_copy(zn, zb[:])

                # ---- S_{j-1} and c_{j-1} (one iteration late, off-chain) ----
                if 2 <= j <= n_steps - CLAG + 1:
                    ss = spool.tile([128, W], F16, tag="ss", name="ss")
                    nc.vector.tensor_scalar(ss[:], q_jm1[:], -1.0, 1.0,
                                            op0=ALU.mult, op1=ALU.add)
                    cc = cpool.tile([128, W], F16, tag="cc", name="cc")
                    nc.vector.scalar_tensor_tensor(
                        cc[:], db_jm1[:], 1.0, ss[:],
                        op0=ALU.mult, op1=ALU.mult)
                    cql.append(cc)

                if s == blk - 1:
                    nc.sync.dma_start(OUT[:, t0i:t0i + blk, :],
                                      stg[:, 0:blk, :])

                z_m2 = z
                z = zn
                t_m2 = t_jm1
                t_jm1 = t_j
                q_jm1 = q_j
                db_jm1 = db
    nc.compile()
    return nc


def kernel(X0, MA, MW, bA_z, bW_z, by_w):
    global LAST_RESULT
    from concourse.bass_utils import run_bass_kernel_spmd

    X0 = np.asarray(X0, dtype=np.float32)
    MA = np.asarray(MA, dtype=np.float32)
    MW = np.asarray(MW, dtype=np.float32)
    bA_z = np.asarray(bA_z, dtype=np.float32)
    bW_z = np.asarray(bW_z, dtype=np.float32)
    by_w = np.asarray(by_w, dtype=np.float32)

    bA = np.float32(0.5) * np.exp(-bA_z[0, 0] * bA_z[0, 0]) + np.float32(0.5)
    bW = np.float32(0.5) * np.exp(-bW_z[0, 0] * bW_z[0, 0]) + np.float32(0.5)
    I = np.eye(N, dtype=np.float32)
    A = (1 - bA) * (MA + MA.T) + bA * (MA - MA.T) - np.float32(YA) * I
    C = (1 - bA) * (MW + MW.T) + bW * (MW - MW.T) - np.float32(YW) * I

    f16 = lambda x: x.astype(np.float16).astype(np.float32)
    Ah = f16(np.float32(STEP) * A)
    Cs = f16(np.float32(STEP) * C)
    Dm = I + np.float32(STEP) * A
    CsD = f16((np.float32(STEP) * C) @ Dm)
    CsD2 = f16((np.float32(STEP) * C) @ Dm @ Dm)
    byh = f16(by_w)

    NPACK = 19 * 128 + BLOC + 4 * BLOC
    base = np.zeros((128, NPACK), dtype=np.float16)
    for wi, M in enumerate([Ah, Cs, CsD, CsD2]):
        MT = M.T.astype(np.float16)
        for k in range(2):
            for m in range(2):
                idx = 4 * wi + 2 * k + m
                base[:, idx * 128:(idx + 1) * 128] = \
                    MT[128 * k:128 * (k + 1), 128 * m:128 * (m + 1)]
    base[0, 16 * 128:16 * 128 + 128] = byh[0:128, 0].astype(np.float16)
    base[0, 17 * 128:17 * 128 + 128] = byh[128:256, 0].astype(np.float16)
    base[:, 18 * 128:19 * 128] = np.eye(128, dtype=np.float16)
    base[0, 2432:2432 + BLOC] = 1.0

    in_maps = []
    for i in range(NCORES):
        Xc = X0[i * BLOC:(i + 1) * BLOC, :] / np.float32(STEP)   # [32, 256]
        z0 = Xc.T.astype(np.float16)                              # [256, 32]
        z0f = z0.astype(np.float32)
        T0f = np.tanh(Cs @ z0f + byh).astype(np.float16)          # [256, 32]
        WPACKh = base.copy()
        for c in range(2):
            WPACKh[:, 2464 + c * BLOC:2464 + (c + 1) * BLOC] = \
                z0[c * 128:(c + 1) * 128, :]
            WPACKh[:, 2528 + c * BLOC:2528 + (c + 1) * BLOC] = \
                T0f[c * 128:(c + 1) * 128, :]
        in_maps.append({"WPACK": WPACKh})

    nc = _build(NSTEPS)
    res = run_bass_kernel_spmd(nc, in_maps, core_ids=list(range(NCORES)))
    LAST_RESULT = res

    out = np.empty((BS, TMAX, N), dtype=np.float32)
    out[:, 0, :] = X0
    for i in range(NCORES):
        O = np.asarray(res.results[i]["OUT"]).reshape(128, NSTEPS, 2, BLOC)
        # [p, t, c, b] -> [b, t, c, p] -> [32, 511, 256]
        blockX = O.transpose(3, 1, 2, 0).reshape(BLOC, NSTEPS, N)
        out[i * BLOC:(i + 1) * BLOC, 1:, :] = (
            blockX.astype(np.float32) * np.float32(STEP))
    return out


if __name__ == "__main__":
    rng = np.random.default_rng(0)
    inputs = {
        "X0": rng.standard_normal((BS, N), dtype=np.float32),
        "MA": rng.standard_normal((N, N), dtype=np.float32) / 16,
        "MW": rng.standard_normal((N, N), dtype=np.float32) / 16,
        "bA_z": np.full((1, 1), 0.65, dtype=np.float32),
        "bW_z": np.full((1, 1), 0.65, dtype=np.float32),
        "by_w": rng.standard_normal((N, 1), dtype=np.float32) / 100,
    }
    out = kernel(**inputs)
    print("out", out.shape, out.dtype, np.abs(out).max())
